# revision 8
# baseline (speedup 1.0000x reference)
"""GATv2 (2-layer) + mean-pool + linear on 8 Trainium2 NeuronCores.

Sharding: edges partitioned by dst-node range (edge parallel, owner-computes
aggregation). Core k owns nodes [k*SH, (k+1)*SH) and every edge targeting
them, so segment softmax/aggregation are core-local; only small per-node
layer-2 tables and the pooled output cross cores (AllGather / AllReduce).

Self-contained: hardcodes the problem shapes from setup_inputs().
"""

import numpy as np

import concourse.bacc as bacc
import concourse.bass as bass
import concourse.mybir as mybir
import concourse.tile as tile

F32 = mybir.dt.float32
I32 = mybir.dt.int32
AF = mybir.ActivationFunctionType
ALU = mybir.AluOpType

CFG = dict(N=50000, E=800000, G=128, NC=8)
WN = 125          # nodes per window
SPAN = 16         # max dst-span of one 128-edge subchunk
OFFMAX = WN - SPAN
P = 128

_CACHE = {}


# ----------------------------------------------------------------------------
# host-side preprocessing
# ----------------------------------------------------------------------------

BK = 64                    # bucket size (PE col-group aligned: 0 or 64)
NB = -(-WN // BK)          # buckets per window


def _bucket_bounds():
    return [min(BK * b, WN) for b in range(NB + 1)]


def _host_prep(inputs, cfg):
    N, E, G, NC = cfg["N"], cfg["E"], cfg["G"], cfg["NC"]
    SH = N // NC
    NW = SH // WN
    SHPAD = ((SH + 127) // 128) * 128
    NPAD = SHPAD * NC
    XRROWS = SHPAD + 128

    ei = np.asarray(inputs["edge_index"]).astype(np.int64)
    ea = np.asarray(inputs["edge_attr"]).astype(np.float32)[:, 0]
    batch = np.asarray(inputs["batch"]).astype(np.int64)
    src_all, dst_all = ei[0], ei[1]

    cnt = np.bincount(dst_all, minlength=N).astype(np.float32)
    ea_sum = np.bincount(dst_all, weights=ea, minlength=N).astype(np.float32)
    loop_ea = ea_sum / np.maximum(cnt, 1.0)

    srcx = np.concatenate([src_all, np.arange(N, dtype=np.int64)])
    dstx = np.concatenate([dst_all, np.arange(N, dtype=np.int64)])
    eax = np.concatenate([ea, loop_ea]).astype(np.float32)
    order = np.argsort(dstx, kind="stable")
    srcx, dstx, eax = srcx[order], dstx[order], eax[order]

    # window boundaries (global, NW*NC windows of WN nodes)
    wb = np.searchsorted(dstx, np.arange(0, N + 1, WN))

    # per (core, window): relative dst + per-bucket subchunk demand
    rels = {}
    bb = _bucket_bounds()
    needC = np.zeros((NC, NW, NB), np.int64)
    for k in range(NC):
        for w in range(NW):
            gw = k * NW + w
            s, e = wb[gw], wb[gw + 1]
            rel = (dstx[s:e] - (k * SH + w * WN)).astype(np.int64)
            cuts = np.searchsorted(rel, bb)
            rels[(k, w)] = (s, rel, cuts)
            for b in range(NB):
                needC[k, w, b] = -(-(cuts[b + 1] - cuts[b]) // P)
    Cb = [[int(needC[:, w, b].max()) for b in range(NB)] for w in range(NW)]
    Cs = [int(sum(Cb[w])) for w in range(NW)]
    TOTC = int(sum(Cs))
    cum = np.concatenate([[0], np.cumsum(Cs)]).astype(int)
    # per-window flat subchunk -> bucket (PSUM partition offset = BK*bucket)
    subbuck = [[b for b in range(NB) for _ in range(Cb[w][b])]
               for w in range(NW)]

    gcnt = np.bincount(batch, minlength=G).astype(np.float32)
    recip_cnt = (1.0 / np.maximum(gcnt, 1.0)).astype(np.float32)

    per_core = []
    for k in range(NC):
        gsrc = np.zeros((P, TOTC), np.int32)
        gdstl = np.zeros((P, TOTC), np.int32)
        g2s = np.zeros((P, TOTC), np.int32)
        g2d = np.zeros((P, TOTC), np.int32)
        ea_sl = np.zeros((P, TOTC), np.float32)
        oh16 = np.zeros((P, TOTC, BK), np.float32)
        for w in range(NW):
            s, rel, cuts = rels[(k, w)]
            cc = cum[w]
            for b in range(NB):
                a0 = cuts[b]
                for j in range(Cb[w][b]):
                    a = a0 + j * P
                    e_ = min(a0 + (j + 1) * P, cuts[b + 1])
                    npk = max(0, e_ - a)
                    if npk > 0:
                        sl = slice(s + a, s + e_)
                        gsrc[0:npk, cc] = srcx[sl]
                        gdstl[0:npk, cc] = dstx[sl] - k * SH
                        g2s[0:npk, cc] = (SHPAD * (srcx[sl] // SH)
                                          + srcx[sl] % SH)
                        g2d[0:npk, cc] = (SHPAD * (dstx[sl] // SH)
                                          + dstx[sl] % SH)
                        ea_sl[0:npk, cc] = eax[sl]
                        oh16[np.arange(npk), cc, rel[a:e_] - BK * b] = 1.0
                    cc += 1
        ea_row = ea_sl.T.reshape(1, TOTC * P).copy()

        poh = np.zeros((BK, NW * NB, P), np.float32)
        gb = batch[k * SH + np.arange(SH)]
        for w in range(NW):
            for b in range(NB):
                m = min(BK, WN - BK * b)
                gg = gb[w * WN + BK * b:w * WN + BK * b + m]
                poh[np.arange(m), w * NB + b, gg] = recip_cnt[gg]

        per_core.append(dict(
            gsrc=gsrc, gdstl=gdstl, g2s=g2s, g2d=g2d,
            ea_sl=ea_sl, ea_row=ea_row,
            oh16=oh16.reshape(P, TOTC * BK).copy(),
            poh=poh.reshape(BK, NW * NB * P).copy(),
        ))

    # ---- weights / constants ----
    f = lambda n: np.asarray(inputs[n]).astype(np.float32)
    Wl1, bl1, Wr1, br1 = f("Wl1"), f("bl1"), f("Wr1"), f("br1")
    We1, att1, bias1 = f("We1"), f("att1"), f("bias1")
    Wl2, bl2, Wr2, br2 = f("Wl2"), f("bl2"), f("Wr2"), f("br2")
    We2, att2, bias2 = f("We2"), f("att2"), f("bias2")
    W3, b3, x = f("W3"), f("b3"), f("x")

    H1, C1 = att1.shape
    D1 = H1 * C1
    DIN = x.shape[1]
    attb = np.zeros((D1, H1), np.float32)
    for h in range(H1):
        attb[h * C1:(h + 1) * C1, h] = att1[h]

    xT1raw = np.zeros((DIN + 1, NPAD), np.float32)
    xT1raw[:DIN, :N] = x.T
    xT1raw[DIN, :] = 1.0

    Wlb1 = np.concatenate([Wl1, bl1[None, :]], 0)
    Wrb1 = np.concatenate([Wr1, br1[None, :]], 0)
    Wfull1l = np.concatenate([Wlb1, 0.2 * (Wlb1 @ attb)], 1)
    Wfull1r = np.concatenate([Wrb1, 0.2 * (Wrb1 @ attb)], 1)
    att2v = att2[0]
    D2 = Wl2.shape[1]
    Wlr2aug = np.concatenate(
        [Wl2, Wr2, 0.2 * (Wl2 @ att2v)[:, None], 0.2 * (Wr2 @ att2v)[:, None]], 1)
    b2row = np.concatenate(
        [bl2, br2, 0.2 * np.array([bl2 @ att2v]), 0.2 * np.array([br2 @ att2v])])

    XT1W = SHPAD + XRROWS
    consts = dict(
        Wfull1l=Wfull1l, Wfull1r=Wfull1r,
        attb08=(0.8 * attb),
        r02t=np.tile((0.2 * (We1[0] @ attb))[None, :], (P, 1)),
        we1row=We1[0:1, :].copy(),
        Wlr2aug=Wlr2aug,
        b2tab=np.tile(b2row[None, :], (P, 1)),
        we2t=np.tile(We2[0:1, :], (P, 1)),
        att2t08=np.tile(0.8 * att2v[None, :], (P, 1)),
        bias2rep=np.tile(bias2[None, :], (P, 1)),
        bias1rep=np.tile(bias1[None, :], (P, 1)),
        ident=np.eye(P, dtype=np.float32),
        W3t=W3.copy(),
        b3t=np.tile(b3[None, :], (P, 1)),
        zrow=np.zeros((1, P), np.float32),
    )
    r2s02 = float(0.2 * (att2v @ We2[0]))

    meta = dict(Cs=Cs, Cb=Cb, subbuck=subbuck, TOTC=TOTC,
                cum=[int(v) for v in cum], NW=NW, SH=SH,
                SHPAD=SHPAD, NPAD=NPAD, XRROWS=XRROWS, XT1W=XT1W,
                r2s02=r2s02, NC=NC, G=G, N=N, DIN=DIN, D1=D1, H1=H1,
                D2=D2, xT1raw=None)
    extras = dict(xT1raw=xT1raw)
    return meta, per_core, consts, extras


# ----------------------------------------------------------------------------
# bass program
# ----------------------------------------------------------------------------

def _build(meta):
    NC, G, NW = meta["NC"], meta["G"], meta["NW"]
    SH, SHPAD = meta["SH"], meta["SHPAD"]
    XRROWS, XT1W = meta["XRROWS"], meta["XT1W"]
    Cs, TOTC, cum = meta["Cs"], meta["TOTC"], meta["cum"]
    DIN, D1, H1, D2 = meta["DIN"], meta["D1"], meta["H1"], meta["D2"]
    DA = D1 + H1          # 136  (feat + 0.2*att-projected)
    DB = 2 * D2 + 2       # 18
    nxl, nxr = SHPAD // P, XRROWS // P

    nc = bacc.Bacc("TRN2", target_bir_lowering=False, debug=False,
                   num_devices=NC)

    inp = {}
    for name, shape, dt in [
        ("xT1", [DIN + 1, XT1W], F32),
        ("Wfull1l", [DIN + 1, DA], F32), ("Wfull1r", [DIN + 1, DA], F32),
        ("attb08", [D1, H1], F32), ("r02t", [P, H1], F32),
        ("we1row", [1, D1], F32),
        ("Wlr2aug", [D1, DB], F32), ("b2tab", [P, DB], F32),
        ("we2t", [P, D2], F32), ("att2t08", [P, D2], F32),
        ("bias2rep", [P, D2], F32), ("bias1rep", [P, D1], F32),
        ("ident", [P, P], F32), ("W3t", [D2, 1], F32), ("b3t", [P, 1], F32),
        ("zrow", [1, P], F32),
        ("gsrc", [P, TOTC], I32), ("gdstl", [P, TOTC], I32),
        ("g2s", [P, TOTC], I32), ("g2d", [P, TOTC], I32),
        ("ea_sl", [P, TOTC], F32), ("ea_row", [1, TOTC * P], F32),
        ("oh16", [P, TOTC * BK], F32), ("poh", [BK, NW * NB * P], F32),
    ]:
        inp[name] = nc.dram_tensor(name, shape, dt, kind="ExternalInput")
    out_d = nc.dram_tensor("out", [G, 1], F32, kind="ExternalOutput")

    rg = [list(range(NC))]

    with tile.TileContext(nc) as tc:
        with (
            tc.tile_pool(name="dram", bufs=1, space="DRAM") as dram,
            tc.tile_pool(name="cst", bufs=1) as cst,
            tc.tile_pool(name="sm", bufs=3) as sm,
        ):
            XLsh = dram.tile([SHPAD, DA], F32)
            XLfull = dram.tile([NC * SHPAD, DA], F32)
            XR = dram.tile([XRROWS, DA], F32)
            X2sh = dram.tile([SHPAD, DB], F32)
            X2full = dram.tile([NC * SHPAD, DB], F32)
            ARin = dram.tile([G, D2], F32)
            ARout = dram.tile([G, D2], F32)

            c_t = {}
            for name in ["Wfull1l", "Wfull1r", "attb08", "r02t", "we1row",
                         "Wlr2aug", "b2tab", "we2t", "att2t08", "bias2rep",
                         "bias1rep", "ident", "W3t", "b3t", "zrow"]:
                t = cst.tile(list(inp[name].shape), F32, tag=name)
                nc.sync.dma_start(t[:], inp[name][:])
                c_t[name] = t
            idx_t = {}
            for name in ["gsrc", "gdstl", "g2s", "g2d"]:
                t = cst.tile([P, TOTC], I32, tag=name)
                nc.sync.dma_start(t[:], inp[name][:])
                idx_t[name] = t
            easl_t = cst.tile([P, TOTC], F32, tag="easl")
            nc.sync.dma_start(easl_t[:], inp["ea_sl"][:])

            # ---------------- phase 0: node tables ----------------
            with (
                tc.tile_pool(name="p0ps", bufs=2, space="PSUM") as p0ps,
                tc.tile_pool(name="p0sb", bufs=3) as p0sb,
            ):
                xslab = cst.tile([DIN + 1, SHPAD], F32, tag="xslab")
                nc.sync.dma_start(xslab[:], inp["xT1"][:, 0:SHPAD])
                xslab2 = cst.tile([DIN + 1, XRROWS], F32, tag="xslab2")
                nc.sync.dma_start(xslab2[:], inp["xT1"][:, SHPAD:SHPAD + XRROWS])

                for t in range(nxl):
                    ps = p0ps.tile([P, DA], F32, tag="tab")
                    nc.tensor.matmul(ps[:], xslab[:, t * P:(t + 1) * P],
                                     c_t["Wfull1l"][:], start=True, stop=True)
                    s = p0sb.tile([P, DA], F32, tag="tabs")
                    nc.scalar.activation(s[:], ps[:], AF.Copy)
                    nc.sync.dma_start(XLsh[t * P:(t + 1) * P, :], s[:])
                for t in range(nxr):
                    ps = p0ps.tile([P, DA], F32, tag="tab")
                    nc.tensor.matmul(ps[:], xslab2[:, t * P:(t + 1) * P],
                                     c_t["Wfull1r"][:], start=True, stop=True)
                    s = p0sb.tile([P, DA], F32, tag="tabs")
                    nc.scalar.activation(s[:], ps[:], AF.Copy)
                    nc.sync.dma_start(XR[t * P:(t + 1) * P, :], s[:])

            nc.gpsimd.collective_compute(
                "AllGather", ALU.bypass, replica_groups=rg,
                ins=[XLsh.opt()], outs=[XLfull.opt()])

            # ---------------- phase 1: layer-1 windows ----------------
            with (
                tc.tile_pool(name="p1u", bufs=2, space="PSUM") as p1u,
                tc.tile_pool(name="p1l", bufs=1, space="PSUM") as p1l,
                tc.tile_pool(name="p1a", bufs=1, space="PSUM") as p1a,
                tc.tile_pool(name="p1h", bufs=1, space="PSUM") as p1h,
                tc.tile_pool(name="p1w", bufs=2) as p1w,
            ):
                for w in range(NW):
                    C, c0 = Cs[w], cum[w]

                    xl_g = p1w.tile([P, C, DA], F32, tag="xl_g")
                    xr_g = p1w.tile([P, C, DA], F32, tag="xr_g")
                    for c in range(C):
                        nc.gpsimd.indirect_dma_start(
                            out=xl_g[:, c, :], out_offset=None, in_=XLfull[:],
                            in_offset=bass.IndirectOffsetOnAxis(
                                ap=idx_t["gsrc"][:, c0 + c:c0 + c + 1], axis=0))
                        nc.gpsimd.indirect_dma_start(
                            out=xr_g[:, c, :], out_offset=None, in_=XR[:],
                            in_offset=bass.IndirectOffsetOnAxis(
                                ap=idx_t["gdstl"][:, c0 + c:c0 + c + 1], axis=0))
                    earow = p1w.tile([1, C * P], F32, tag="earow")
                    nc.sync.dma_start(earow[:],
                                      inp["ea_row"][:, c0 * P:(c0 + C) * P])
                    oh = p1w.tile([P, C, BK], F32, tag="oh")
                    nc.sync.dma_start(
                        oh[:], inp["oh16"][:, c0 * BK:(c0 + C) * BK])

                    aug = p1w.tile([P, C, DA], F32, tag="aug")

                    for ch0 in range(0, C, 4):
                        ns = min(4, C - ch0)
                        ups = p1u.tile([P, 512], F32, tag="ups")
                        nc.tensor.matmul(
                            ups[:, 0:ns * P], c_t["we1row"][:],
                            earow[:, ch0 * P:(ch0 + ns) * P],
                            start=True, stop=False)
                        for s in range(ns):
                            for g_t in (xl_g, xr_g):
                                nc.tensor.matmul(
                                    ups[:, s * P:(s + 1) * P],
                                    g_t[:, ch0 + s, 0:D1], c_t["ident"][:],
                                    is_transpose=True, start=False,
                                    stop=(s == ns - 1 and g_t is xr_g))
                        m_sb = sm.tile([P, 512], F32, tag="m_sb")
                        nc.scalar.activation(m_sb[:, 0:ns * P],
                                             ups[:, 0:ns * P], AF.Relu)
                        ltr = p1l.tile([P, 4 * H1], F32, tag="ltr")
                        for s in range(ns):
                            nc.tensor.matmul(
                                ltr[:, s * H1:(s + 1) * H1],
                                m_sb[:, s * P:(s + 1) * P], c_t["attb08"][:],
                                start=(s == 0), stop=(s == ns - 1))
                        t1 = sm.tile([P, 4, H1], F32, tag="t1")
                        nc.vector.tensor_add(
                            t1[:, 0:ns, :], xl_g[:, ch0:ch0 + ns, D1:DA],
                            xr_g[:, ch0:ch0 + ns, D1:DA])
                        er = sm.tile([P, 4, H1], F32, tag="er")
                        nc.vector.tensor_tensor(
                            er[:, 0:ns, :],
                            in0=easl_t[:, c0 + ch0:c0 + ch0 + ns, None]
                                .to_broadcast([P, ns, H1]),
                            in1=c_t["r02t"][:, None, :].to_broadcast(
                                [P, ns, H1]),
                            op=ALU.mult)
                        nc.vector.tensor_add(
                            t1[:, 0:ns, :], t1[:, 0:ns, :],
                            ltr[:, 0:ns * H1].rearrange(
                                "p (s h) -> p s h", h=H1))
                        nc.vector.tensor_add(t1[:, 0:ns, :], t1[:, 0:ns, :],
                                             er[:, 0:ns, :])
                        nc.scalar.activation(aug[:, ch0:ch0 + ns, D1:DA],
                                             t1[:, 0:ns, :], AF.Exp)

                    nc.vector.tensor_tensor(
                        aug[:, :, 0:D1].rearrange(
                            "p c (h x) -> p c h x", h=H1),
                        in0=xl_g[:, :, 0:D1].rearrange(
                            "p c (h x) -> p c h x", h=H1),
                        in1=aug[:, :, D1:DA][:, :, :, None]
                            .to_broadcast([P, C, H1, D1 // H1]),
                        op=ALU.mult)

                    # per-bucket aggregation (base partition 0 everywhere)
                    Cb = meta["Cb"][w]
                    htp = p1h.tile([P, P], F32, tag="htp")
                    ci = 0
                    for b in range(NB):
                        if Cb[b] == 0:
                            continue
                        m = min(BK, WN - BK * b)
                        ab = p1a.tile([BK, DA], F32, tag=f"agg{b}")
                        for j in range(Cb[b]):
                            nc.tensor.matmul(
                                ab[:], oh[:, ci, :], aug[:, ci, :],
                                start=(j == 0), stop=(j == Cb[b] - 1))
                            ci += 1
                        # drain: normalize + bias + ELU (per bucket)
                        rden = sm.tile([BK, H1], F32, tag="rden")
                        nc.vector.reciprocal(rden[0:m, :], ab[0:m, D1:DA])
                        hsb = sm.tile([BK, D1], F32, tag="hsb")
                        nc.vector.tensor_tensor(
                            hsb[0:m, :].rearrange("p (h x) -> p h x", h=H1),
                            in0=ab[0:m, 0:D1].rearrange(
                                "p (h x) -> p h x", h=H1),
                            in1=rden[0:m, :, None].to_broadcast(
                                [m, H1, D1 // H1]),
                            op=ALU.mult)
                        nc.vector.tensor_add(hsb[0:m, :], hsb[0:m, :],
                                             c_t["bias1rep"][0:m, :])
                        mn = sm.tile([BK, D1], F32, tag="mn")
                        nc.vector.tensor_scalar(
                            out=mn[0:m, :], in0=hsb[0:m, :], scalar1=0.0,
                            scalar2=None, op0=ALU.min)
                        nc.scalar.activation(mn[0:m, :], mn[0:m, :], AF.Exp)
                        nc.vector.tensor_scalar(
                            out=mn[0:m, :], in0=mn[0:m, :], scalar1=1.0,
                            scalar2=None, op0=ALU.subtract)
                        nc.vector.tensor_max(hsb[0:m, :], hsb[0:m, :],
                                             mn[0:m, :])
                        nc.tensor.matmul(htp[:, BK * b:BK * b + m],
                                         hsb[0:m, :], c_t["ident"][0:m, 0:m],
                                         is_transpose=True, start=(b == 0),
                                         stop=(b == NB - 1))

                    # layer-2 per-node table: [xl2|xr2|p2|q2] = h @ Wlr2aug
                    hts = sm.tile([P, P], F32, tag="hts")
                    nc.scalar.activation(hts[:, 0:WN], htp[:, 0:WN], AF.Copy)
                    x2p = p1h.tile([P, DB], F32, tag="x2p")
                    nc.tensor.matmul(x2p[0:WN, :], hts[:, 0:WN],
                                     c_t["Wlr2aug"][:], start=True, stop=True)
                    x2s = sm.tile([P, DB], F32, tag="x2s")
                    nc.vector.tensor_add(x2s[0:WN, :], x2p[0:WN, :],
                                         c_t["b2tab"][0:WN, :])
                    nc.sync.dma_start(X2sh[w * WN:(w + 1) * WN, :],
                                      x2s[0:WN, :])

                if SHPAD > SH:
                    zt = sm.tile([P, DB], F32, tag="x2s")
                    nc.vector.memset(zt[:], 0.0)
                    nc.sync.dma_start(X2sh[SH:SHPAD, :], zt[0:SHPAD - SH, :])

            nc.gpsimd.collective_compute(
                "AllGather", ALU.bypass, replica_groups=rg,
                ins=[X2sh.opt()], outs=[X2full.opt()])

            # ---------------- phase 2: layer-2 windows + pooling ----------
            with (
                tc.tile_pool(name="p2o", bufs=2, space="PSUM") as p2o,
                tc.tile_pool(name="p2p", bufs=1, space="PSUM") as p2p,
                tc.tile_pool(name="p2w", bufs=2) as p2w,
            ):
                poolp = p2p.tile([P, D2], F32, tag="pool")
                for w in range(NW):
                    C, c0 = Cs[w], cum[w]
                    g2s_t = p2w.tile([P, C, DB], F32, tag="g2s_t")
                    g2d_t = p2w.tile([P, C, DB], F32, tag="g2d_t")
                    for c in range(C):
                        nc.gpsimd.indirect_dma_start(
                            out=g2s_t[:, c, :], out_offset=None, in_=X2full[:],
                            in_offset=bass.IndirectOffsetOnAxis(
                                ap=idx_t["g2s"][:, c0 + c:c0 + c + 1], axis=0))
                        nc.gpsimd.indirect_dma_start(
                            out=g2d_t[:, c, :], out_offset=None, in_=X2full[:],
                            in_offset=bass.IndirectOffsetOnAxis(
                                ap=idx_t["g2d"][:, c0 + c:c0 + c + 1], axis=0))
                    oh = p2w.tile([P, C, 32], F32, tag="oh2")
                    nc.sync.dma_start(
                        oh[:], inp["oh16"][:, c0 * BK:(c0 + C) * BK])

                    u2 = p2w.tile([P, C, D2], F32, tag="u2")
                    nc.vector.tensor_add(u2[:], g2s_t[:, :, 0:D2],
                                         g2d_t[:, :, D2:2 * D2])
                    ee2 = p2w.tile([P, C, D2], F32, tag="ee2")
                    nc.vector.tensor_tensor(
                        ee2[:],
                        in0=easl_t[:, c0:c0 + C, None].to_broadcast(
                            [P, C, D2]),
                        in1=c_t["we2t"][:, None, :].to_broadcast([P, C, D2]),
                        op=ALU.mult)
                    nc.vector.tensor_add(u2[:], u2[:], ee2[:])
                    r2 = p2w.tile([P, C, D2], F32, tag="r2")
                    nc.scalar.activation(r2[:], u2[:], AF.Relu)
                    nc.vector.tensor_tensor(
                        r2[:], in0=r2[:],
                        in1=c_t["att2t08"][:, None, :].to_broadcast(
                            [P, C, D2]),
                        op=ALU.mult)
                    lg = p2w.tile([P, C, 1], F32, tag="lg")
                    nc.vector.tensor_reduce(lg[:], r2[:],
                                            axis=mybir.AxisListType.X,
                                            op=ALU.add)
                    nc.vector.tensor_add(lg[:], lg[:],
                                         g2s_t[:, :, 2 * D2:2 * D2 + 1])
                    nc.vector.tensor_add(lg[:], lg[:],
                                         g2d_t[:, :, 2 * D2 + 1:2 * D2 + 2])
                    er2 = p2w.tile([P, C, 1], F32, tag="er2")
                    nc.vector.tensor_scalar(
                        out=er2[:], in0=easl_t[:, c0:c0 + C, None],
                        scalar1=meta["r2s02"], scalar2=None, op0=ALU.mult)
                    nc.vector.tensor_add(lg[:], lg[:], er2[:])
                    aug2 = p2w.tile([P, C, D2 + 1], F32, tag="aug2")
                    nc.scalar.activation(aug2[:, :, 0:1], lg[:], AF.Exp)
                    nc.vector.tensor_tensor(
                        aug2[:, :, 1:D2 + 1],
                        in0=aug2[:, :, 0:1].to_broadcast([P, C, D2]),
                        in1=g2s_t[:, :, 0:D2], op=ALU.mult)

                    Cb = meta["Cb"][w]
                    ci = 0
                    for b in range(NB):
                        if Cb[b] == 0:
                            continue
                        m = min(BK, WN - BK * b)
                        o2 = p2o.tile([BK, D2 + 1], F32, tag=f"o2{b}")
                        for j in range(Cb[b]):
                            nc.tensor.matmul(
                                o2[:], oh[:, ci, :], aug2[:, ci, :],
                                start=(j == 0), stop=(j == Cb[b] - 1))
                            ci += 1
                        rden2 = sm.tile([BK, 1], F32, tag="rden2")
                        nc.vector.reciprocal(rden2[0:m, :], o2[0:m, 0:1])
                        h2 = sm.tile([BK, D2], F32, tag="h2")
                        nc.vector.tensor_scalar(
                            out=h2[0:m, :], in0=o2[0:m, 1:D2 + 1],
                            scalar1=rden2[0:m, :], scalar2=None, op0=ALU.mult)
                        nc.vector.tensor_add(h2[0:m, :], h2[0:m, :],
                                             c_t["bias2rep"][0:m, :])
                        mn2 = sm.tile([BK, D2], F32, tag="mn2")
                        nc.vector.tensor_scalar(
                            out=mn2[0:m, :], in0=h2[0:m, :], scalar1=0.0,
                            scalar2=None, op0=ALU.min)
                        nc.scalar.activation(mn2[0:m, :], mn2[0:m, :], AF.Exp)
                        nc.vector.tensor_scalar(
                            out=mn2[0:m, :], in0=mn2[0:m, :], scalar1=1.0,
                            scalar2=None, op0=ALU.subtract)
                        nc.vector.tensor_max(h2[0:m, :], h2[0:m, :],
                                             mn2[0:m, :])

                        pohw = p2w.tile([BK, P], F32, tag="pohw")
                        nc.sync.dma_start(
                            pohw[:],
                            inp["poh"][:, (w * NB + b) * P:(w * NB + b + 1) * P])
                        nc.tensor.matmul(
                            poolp[:], pohw[0:m, :], h2[0:m, :],
                            start=(w == 0 and b == 0),
                            stop=(w == NW - 1 and b == NB - 1))

                pools = sm.tile([P, D2], F32, tag="pools")
                nc.vector.tensor_copy(pools[:], poolp[:])
                nc.sync.dma_start(ARin[:], pools[0:G, :])

            nc.gpsimd.collective_compute(
                "AllReduce", ALU.add, replica_groups=rg,
                ins=[ARin.opt()], outs=[ARout.opt()])

            with tc.tile_pool(name="fin", bufs=1, space="PSUM") as fin:
                pooled = sm.tile([P, D2], F32, tag="pooled")
                nc.sync.dma_start(pooled[0:G, :], ARout[:])
                ptp = fin.tile([D2, P], F32, tag="ptp")
                nc.tensor.matmul(ptp[:, 0:G], pooled[0:G, :],
                                 c_t["ident"][0:G, 0:G],
                                 is_transpose=True, start=True, stop=True)
                pts = sm.tile([D2, P], F32, tag="pts")
                nc.scalar.activation(pts[:, 0:G], ptp[:, 0:G], AF.Copy)
                fps = fin.tile([P, 1], F32, tag="fps")
                nc.tensor.matmul(fps[0:G, :], pts[:, 0:G], c_t["W3t"][:],
                                 start=True, stop=True)
                fsb = sm.tile([P, 1], F32, tag="fsb")
                nc.vector.tensor_add(fsb[0:G, :], fps[0:G, :],
                                     c_t["b3t"][0:G, :])
                nc.sync.dma_start(out_d[:], fsb[0:G, :])

    nc.compile()
    return nc


def _in_maps(meta, per_core, consts, extras):
    NC, SH, SHPAD = meta["NC"], meta["SH"], meta["SHPAD"]
    XRROWS, XT1W = meta["XRROWS"], meta["XT1W"]
    xT1raw = extras["xT1raw"]
    W = xT1raw.shape[1]

    def cols(start, width):
        out = np.zeros((xT1raw.shape[0], width), np.float32)
        end = min(start + width, W)
        if end > start:
            out[:, 0:end - start] = xT1raw[:, start:end]
        return out

    maps = []
    for k in range(NC):
        m = dict(per_core[k])
        m.update(consts)
        xt = np.zeros((xT1raw.shape[0], XT1W), np.float32)
        xt[:, 0:SHPAD] = cols(k * SHPAD, SHPAD)
        xt[:, SHPAD:SHPAD + XRROWS] = cols(k * SH, XRROWS)
        m["xT1"] = xt
        maps.append(m)
    return maps


def build_all(inputs, cfg=None):
    cfg = cfg or CFG
    meta, per_core, consts, extras = _host_prep(inputs, cfg)
    key = (tuple(tuple(r) for r in meta["Cb"]), cfg["N"], cfg["E"])
    if key not in _CACHE:
        _CACHE[key] = _build(meta)
    return _CACHE[key], _in_maps(meta, per_core, consts, extras), meta


def kernel(**inputs):
    nc, maps, meta = build_all(inputs, CFG)
    from concourse.bass_utils import run_bass_kernel_spmd
    res = run_bass_kernel_spmd(nc, maps, list(range(CFG["NC"])))
    return np.asarray(res.results[0]["out"], dtype=np.float32)
